# revision 4
# baseline (speedup 1.0000x reference)
"""DenseKAN forward for Trainium2, data-parallel over 8 NeuronCores.

out[b, o] = sum_{i,j} B[b,i,j] * W[i,j,o], where B are cubic B-spline basis
values on a uniform knot grid (linspace(-2.2, 2.2, 12), spacing 0.4).

Math: with s = 2.5*x + 5.5 in [3, 8), every basis is a translate of the
uniform cubic B-spline N3:  B[b,i,j] = N3(s[b,i] - j).  N3 is evaluated
exactly from truncated cubes:
  N3(y) = [relu(z)^3 - 4*relu(z-1)^3] / 6,   z = 2 - |y - 2|
with cheaper one-sided forms for the edge planes (j = 0,1,6,7) where the
reachable range of s only touches one side of the support.

Per core (batch shard 512):
  - x shard is PE-transposed to s_all [128 i-part, (chunk, b) free].
  - For each j: basis plane P_j [128, 4096] (ACT+DVE), then 64 accumulating
    float32r matmuls into 8 PSUM banks that hold the entire [512, 1024]
    output shard.  W is streamed from HBM once (k-order = (j, i)).
"""

import numpy as np

BATCH = 4096
IN_SIZE = 1024
UNITS = 1024
NJ = 8
N_CORES = 8
P = 128
NT = 512  # output n-tile width (one fp32 PSUM bank)

_K2 = float(6.0 ** (-1.0 / 3.0))
_K1 = float((4.0 / 6.0) ** (1.0 / 3.0))


def build_program(nb=BATCH // N_CORES, ni=IN_SIZE, no=UNITS, compile=True):
    import concourse.mybir as mybir
    import concourse.tile as tile
    from concourse import bacc
    from concourse.alu_op_type import AluOpType
    from concourse.masks import make_identity
    from contextlib import ExitStack

    f32 = mybir.dt.float32
    f32r = mybir.dt.float32r
    AF = mybir.ActivationFunctionType

    nchunk = ni // P
    nbt = nb // P
    nnt = no // NT
    free = nchunk * nb
    half = free // 2

    nc = bacc.Bacc("TRN2", target_bir_lowering=False, debug=False)
    x = nc.dram_tensor("x", [nb, ni], f32, kind="ExternalInput").ap()
    w = nc.dram_tensor("w", [ni, NJ, no], f32, kind="ExternalInput").ap()
    out = nc.dram_tensor("out", [nb, no], f32, kind="ExternalOutput").ap()

    with tile.TileContext(nc) as tc, ExitStack() as ctx:
        const = ctx.enter_context(tc.tile_pool(name="const", bufs=1))
        ident = const.tile([P, P], f32)
        make_identity(nc, ident)

        spool = ctx.enter_context(tc.tile_pool(name="spool", bufs=1))
        s_all = spool.tile([P, free], f32)

        # ---- x -> transpose -> s = 2.5*xT + 5.5 ----
        with (
            tc.tile_pool(name="xin", bufs=nbt) as xin,
            tc.tile_pool(name="tpsum", bufs=2, space="PSUM") as tpsum,
        ):
            xts = []
            for bt in range(nbt):
                xt = xin.tile([P, ni], f32, tag="x")
                nc.sync.dma_start(xt[:], x[bt * P : (bt + 1) * P, :])
                xts.append(xt)
            for c in range(nchunk):
                pt = tpsum.tile([P, nb], f32, tag="pt")
                for bt in range(nbt):
                    nc.tensor.transpose(
                        pt[:, bt * P : (bt + 1) * P],
                        xts[bt][:, c * P : (c + 1) * P],
                        ident[:],
                    )
                nc.scalar.activation(
                    s_all[:, c * nb : (c + 1) * nb], pt[:], AF.Copy, bias=5.5, scale=2.5
                )

        tmp = ctx.enter_context(tc.tile_pool(name="tmp", bufs=3))
        ppool = ctx.enter_context(tc.tile_pool(name="pp", bufs=3))
        wpool = ctx.enter_context(tc.tile_pool(name="wp", bufs=6))
        opsum = ctx.enter_context(tc.tile_pool(name="opsum", bufs=1, space="PSUM"))
        outp = ctx.enter_context(tc.tile_pool(name="outp", bufs=2))

        psum_o = {}
        for m in range(nbt):
            for n in range(nnt):
                psum_o[(m, n)] = opsum.tile(
                    [P, NT], f32, tag=f"o{m}_{n}", name=f"po{m}_{n}"
                )

        bias_tiles = {}

        def bias_ap(val):
            val = float(val)
            if val == 0.0:
                return 0.0
            if val not in bias_tiles:
                idx = len(bias_tiles)
                t = const.tile([P, 1], f32, name=f"bias{idx}", tag=f"bias{idx}")
                nc.vector.memset(t[:], val)
                bias_tiles[val] = t
            return bias_tiles[val][:]

        def act(o, i, func, bias=0.0, scale=1.0):
            if func != AF.Copy:
                bias = bias_ap(bias)
            nc.scalar.activation(o[:], i[:], func, bias=bias, scale=scale)

        def vv(o, i0, i1, op):
            nc.vector.tensor_tensor(o[:], i0[:], i1[:], op)

        MUL = AluOpType.mult
        SUB = AluOpType.subtract

        def emit_plane(j, h, ptile):
            sl = slice(h * half, (h + 1) * half)
            s_h = s_all[:, sl]
            p_h = ptile[:, sl]
            cnt = [0]

            def T(tag):
                cnt[0] += 1
                return tmp.tile(
                    [P, half], f32, tag=tag, name=f"t{j}_{h}_{cnt[0]}"
                )
            if j == 0:  # N3(s) on s>=3: (4-s)_+^3 / 6
                r = T("tb")
                act(r, s_h, AF.Relu, scale=-_K2, bias=4 * _K2)
                e = T("ta")
                vv(e, r, r, MUL)
                vv(p_h, e, r, MUL)
            elif j == 7:  # (s-7)_+^3 / 6
                r = T("tb")
                act(r, s_h, AF.Relu, scale=_K2, bias=-7 * _K2)
                e = T("ta")
                vv(e, r, r, MUL)
                vv(p_h, e, r, MUL)
            elif j == 1:  # [(5-s)_+^3 - 4(4-s)_+^3]/6
                r2 = T("tb")
                act(r2, s_h, AF.Relu, scale=-_K2, bias=5 * _K2)
                r1 = T("tc")
                act(r1, s_h, AF.Relu, scale=-_K1, bias=4 * _K1)
                e2 = T("ta")
                act(e2, r2, AF.Square)
                e1 = T("ta")
                act(e1, r1, AF.Square)
                m2 = T("tb")
                vv(m2, e2, r2, MUL)
                m1 = T("tc")
                vv(m1, e1, r1, MUL)
                vv(p_h, m2, m1, SUB)
            elif j == 6:  # [(s-6)_+^3 - 4(s-7)_+^3]/6
                r2 = T("tb")
                act(r2, s_h, AF.Relu, scale=_K2, bias=-6 * _K2)
                r1 = T("tc")
                act(r1, s_h, AF.Relu, scale=_K1, bias=-7 * _K1)
                e2 = T("ta")
                act(e2, r2, AF.Square)
                e1 = T("ta")
                vv(e1, r1, r1, MUL)
                m2 = T("tb")
                vv(m2, e2, r2, MUL)
                m1 = T("tc")
                vv(m1, e1, r1, MUL)
                vv(p_h, m2, m1, SUB)
            else:  # a = |s-(j+2)|; [(2-a)_+^3 - 4(1-a)_+^3]/6
                a = T("ta")
                act(a, s_h, AF.Abs, bias=-(j + 2.0))
                r2 = T("tb")
                act(r2, a, AF.Relu, scale=-_K2, bias=2 * _K2)
                r1 = T("tc")
                act(r1, a, AF.Relu, scale=-_K1, bias=_K1)
                e2 = T("ta")
                act(e2, r2, AF.Square)
                e1 = T("ta")
                act(e1, r1, AF.Square)
                m2 = T("tb")
                vv(m2, e2, r2, MUL)
                m1 = T("tc")
                vv(m1, e1, r1, MUL)
                vv(p_h, m2, m1, SUB)

        for j in range(NJ):
            wts = []
            for c in range(nchunk):
                wt = wpool.tile([P, no], f32r, tag="w")
                nc.sync.dma_start(
                    wt[:], w[c * P : (c + 1) * P, j, :].bitcast(f32r)
                )
                wts.append(wt)
            ptile = ppool.tile([P, free], f32r, tag="p")
            for h in range(2):
                emit_plane(j, h, ptile)
            for c in range(nchunk):
                for m in range(nbt):
                    lhsT = ptile[:, c * nb + m * P : c * nb + (m + 1) * P]
                    for n in range(nnt):
                        nc.tensor.matmul(
                            psum_o[(m, n)][:],
                            lhsT,
                            wts[c][:, n * NT : (n + 1) * NT],
                            start=(j == 0 and c == 0),
                            stop=(j == NJ - 1 and c == nchunk - 1),
                        )

        for m in range(nbt):
            for n in range(nnt):
                ob = outp.tile([P, NT], f32, tag="ob")
                nc.vector.tensor_copy(ob[:], psum_o[(m, n)][:])
                nc.sync.dma_start(out[m * P : (m + 1) * P, n * NT : (n + 1) * NT], ob[:])

    if compile:
        nc.compile()
    return nc


_PROG = None


def _get_prog():
    global _PROG
    if _PROG is None:
        _PROG = build_program()
    return _PROG


def kernel(x, spline_kernel):
    from concourse.bass_utils import run_bass_kernel_spmd

    nc = _get_prog()
    x = np.ascontiguousarray(np.asarray(x, dtype=np.float32))
    w = np.ascontiguousarray(np.asarray(spline_kernel, dtype=np.float32))
    nb = BATCH // N_CORES
    in_maps = [{"x": x[c * nb : (c + 1) * nb], "w": w} for c in range(N_CORES)]
    res = run_bass_kernel_spmd(nc, in_maps, core_ids=list(range(N_CORES)))
    return np.concatenate([res.results[c]["out"] for c in range(N_CORES)], axis=0)


# revision 15
# speedup vs baseline: 369.8775x; 369.8775x over previous
"""DenseKAN forward for Trainium2, data-parallel over 8 NeuronCores.

out[b, o] = sum_{i,j} B[b,i,j] * W[i,j,o], where B are cubic B-spline basis
values on a uniform knot grid (linspace(-2.2, 2.2, 12), spacing 0.4).

Math: with s = 2.5*x + 5.5 in [3, 8), every basis is a translate of the
uniform cubic B-spline N3:  B[b,i,j] = N3(s[b,i] - j).  N3 is evaluated
exactly from truncated cubes:
  N3(y) = [relu(z)^3 - 4*relu(z-1)^3] / 6,   z = 2 - |y - 2|
with cheaper one-sided forms for the edge planes (j = 0,1,6,7) where the
reachable range of s only touches one side of the support.  The affine
s = 2.5*x + 5.5 is folded into the free scale/bias of each plane's first
ScalarE op.

Per core (batch shard 512):
  - x shard is PE-transposed to xT [128 i-part, (chunk, b) free].
  - For each j: basis plane P_j [128, 4096] (ACT+DVE), then 64 accumulating
    float32r matmuls into 8 PSUM banks that hold the entire [512, 1024]
    output shard.  W is streamed from HBM once (k-order = (j, i)).
"""

import numpy as np

BATCH = 4096
IN_SIZE = 1024
UNITS = 1024
NJ = 8
N_CORES = 8
P = 128
NT = 512  # output n-tile width (one fp32 PSUM bank)

_K2 = float(6.0 ** (-1.0 / 3.0))
_K1 = float((4.0 / 6.0) ** (1.0 / 3.0))


def build_program(nb=BATCH // N_CORES, ni=IN_SIZE, no=UNITS, compile=True,
                  repeat=None, tmp_bufs=4, p_bufs=3, w_bufs=6, ablate=None,
                  halves=2, wdtype="f32r", cache_bust=None):
    import concourse.mybir as mybir
    import concourse.tile as tile
    from concourse import bacc
    from concourse.alu_op_type import AluOpType
    from concourse.masks import make_identity
    from contextlib import ExitStack

    f32 = mybir.dt.float32
    f32r = mybir.dt.float32r
    bf16 = mybir.dt.bfloat16
    AF = mybir.ActivationFunctionType
    mmdt = bf16 if wdtype == "bf16" else f32r
    w_dram_dt = bf16 if wdtype == "bf16" else f32

    nchunk = ni // P
    nbt = nb // P
    nnt = no // NT
    free = nchunk * nb
    half = free // halves

    nc = bacc.Bacc("TRN2", target_bir_lowering=False, debug=False)
    x = nc.dram_tensor("x", [nb, ni], f32, kind="ExternalInput").ap()
    w = nc.dram_tensor("w", [ni, NJ, no], w_dram_dt, kind="ExternalInput").ap()
    out = nc.dram_tensor("out", [nb, no], f32, kind="ExternalOutput").ap()

    with tile.TileContext(nc) as tc, ExitStack() as ctx:
        const = ctx.enter_context(tc.tile_pool(name="const", bufs=1))
        ident = const.tile([P, P], f32)
        make_identity(nc, ident)

        bias_tiles = {}

        def bias_ap(val):
            val = float(val)
            if val == 0.0:
                return 0.0
            if val not in bias_tiles:
                idx = len(bias_tiles)
                t = const.tile([P, 1], f32, name=f"bias{idx}", tag=f"bias{idx}")
                nc.vector.memset(t[:], val)
                bias_tiles[val] = t
            return bias_tiles[val][:]

        # pre-register all bias constants (outside any repeat loop)
        for v in (-1.5 * _K2, -0.5 * _K2, -1.5 * _K1, 1.5, 0.5, -0.5, -1.5,
                  2 * _K2, _K1):
            bias_ap(v)
        if cache_bust is not None:
            bias_ap(cache_bust)

        spool = ctx.enter_context(tc.tile_pool(name="spool", bufs=1))
        xT = spool.tile([P, free], f32)

        tmp = ctx.enter_context(tc.tile_pool(name="tmp", bufs=tmp_bufs))
        ppool = ctx.enter_context(tc.tile_pool(name="pp", bufs=p_bufs))
        wpool = ctx.enter_context(tc.tile_pool(name="wp", bufs=w_bufs))
        xin = ctx.enter_context(tc.tile_pool(name="xin", bufs=nbt))
        opsum = ctx.enter_context(tc.tile_pool(name="opsum", bufs=1, space="PSUM"))
        outp = ctx.enter_context(tc.tile_pool(name="outp", bufs=2))

        psum_o = {}
        for m in range(nbt):
            for n in range(nnt):
                psum_o[(m, n)] = opsum.tile(
                    [P, NT], f32, tag=f"o{m}_{n}", name=f"po{m}_{n}"
                )

        def act(o, i, func, bias=0.0, scale=1.0):
            if func != AF.Copy:
                bias = bias_ap(bias)
            nc.scalar.activation(o[:], i[:], func, bias=bias, scale=scale)

        def vv(o, i0, i1, op):
            nc.vector.tensor_tensor(o[:], i0[:], i1[:], op)

        MUL = AluOpType.mult
        SUB = AluOpType.subtract

        def emit_plane(j, h, ptile):
            # reads xT (raw transposed x); s = 2.5*xT + 5.5 folded into the
            # first ScalarE op of each dependency chain.
            sl = slice(h * half, (h + 1) * half)
            s_h = xT[:, sl]
            p_h = ptile[:, sl]
            cnt = [0]

            def T(tag):
                cnt[0] += 1
                return tmp.tile([P, half], f32, tag=tag, name=f"t{j}_{h}_{cnt[0]}")

            if j == 0:  # (4-s)_+^3 / 6
                r = T("tb")
                act(r, s_h, AF.Relu, scale=-2.5 * _K2, bias=-1.5 * _K2)
                e = T("ta")
                vv(e, r, r, MUL)
                vv(p_h, e, r, MUL)
            elif j == 7:  # (s-7)_+^3 / 6
                r = T("tb")
                act(r, s_h, AF.Relu, scale=2.5 * _K2, bias=-1.5 * _K2)
                e = T("ta")
                vv(e, r, r, MUL)
                vv(p_h, e, r, MUL)
            elif j == 1:  # [(5-s)_+^3 - 4(4-s)_+^3]/6
                r2 = T("tb")
                act(r2, s_h, AF.Relu, scale=-2.5 * _K2, bias=-0.5 * _K2)
                r1 = T("tc")
                act(r1, s_h, AF.Relu, scale=-2.5 * _K1, bias=-1.5 * _K1)
                e2 = T("ta")
                act(e2, r2, AF.Square)
                e1 = T("ta")
                act(e1, r1, AF.Square)
                m2 = T("tb")
                vv(m2, e2, r2, MUL)
                m1 = T("tc")
                vv(m1, e1, r1, MUL)
                vv(p_h, m2, m1, SUB)
            elif j == 6:  # [(s-6)_+^3 - 4(s-7)_+^3]/6
                r2 = T("tb")
                act(r2, s_h, AF.Relu, scale=2.5 * _K2, bias=-0.5 * _K2)
                r1 = T("tc")
                act(r1, s_h, AF.Relu, scale=2.5 * _K1, bias=-1.5 * _K1)
                e2 = T("ta")
                act(e2, r2, AF.Square)
                e1 = T("ta")
                vv(e1, r1, r1, MUL)
                m2 = T("tb")
                vv(m2, e2, r2, MUL)
                m1 = T("tc")
                vv(m1, e1, r1, MUL)
                vv(p_h, m2, m1, SUB)
            else:  # a = |s-(j+2)|; [(2-a)_+^3 - 4(1-a)_+^3]/6
                a = T("ta")
                act(a, s_h, AF.Abs, scale=2.5, bias=5.5 - (j + 2.0))
                r2 = T("tb")
                act(r2, a, AF.Relu, scale=-_K2, bias=2 * _K2)
                r1 = T("tc")
                act(r1, a, AF.Relu, scale=-_K1, bias=_K1)
                e2 = T("ta")
                act(e2, r2, AF.Square)
                e1 = T("ta")
                if j == 3:
                    vv(e1, r1, r1, MUL)
                else:
                    act(e1, r1, AF.Square)
                m2 = T("tb")
                vv(m2, e2, r2, MUL)
                m1 = T("tc")
                vv(m1, e1, r1, MUL)
                vv(p_h, m2, m1, SUB)

        wts_pre = None
        if ablate == "now":
            wts_pre = []
            for c in range(nchunk):
                wt = wpool.tile(
                    [P, no], f32r, tag=f"wpre{c}", name=f"wpre{c}", bufs=1
                )
                nc.sync.dma_start(wt[:], w[c * P : (c + 1) * P, 0, :].bitcast(f32r))
                wts_pre.append(wt)

        def body():
            if ablate == "dma":
                for bt in range(nbt):
                    xt = xin.tile([P, ni], f32, tag="x", name=f"xd{bt}")
                    nc.sync.dma_start(xt[:], x[bt * P : (bt + 1) * P, :])
                for j in range(NJ):
                    for c in range(nchunk):
                        wt = wpool.tile([P, no], mmdt, tag="w", name=f"wd{j}_{c}")
                        nc.sync.dma_start(
                            wt[:], w[c * P : (c + 1) * P, j, :].bitcast(mmdt)
                        )
                return
            # ---- x -> PE transpose -> xT (into 2 of the output PSUM banks,
            #      which are free before the accumulation starts) ----
            xts = []
            for bt in range(nbt):
                xt = xin.tile([P, ni], f32, tag="x", name=f"xt{bt}")
                nc.sync.dma_start(xt[:], x[bt * P : (bt + 1) * P, :])
                xts.append(xt)
            for c in range(nchunk):
                pt = psum_o[(0, c % min(2, nnt))]
                for bt in range(nbt):
                    nc.tensor.transpose(
                        pt[:, bt * P : (bt + 1) * P],
                        xts[bt][:, c * P : (c + 1) * P],
                        ident[:],
                    )
                nc.vector.tensor_copy(xT[:, c * nb : (c + 1) * nb], pt[:, : nb])

            for j in range(NJ):
                if wts_pre is not None:
                    wts = wts_pre
                else:
                    wts = []
                    for c in range(nchunk):
                        wt = wpool.tile([P, no], mmdt, tag="w", name=f"w{j}_{c}")
                        nc.sync.dma_start(
                            wt[:], w[c * P : (c + 1) * P, j, :].bitcast(mmdt)
                        )
                        wts.append(wt)
                ptile = ppool.tile([P, free], mmdt, tag="p", name=f"p{j}")
                if ablate == "basis":
                    nc.scalar.activation(
                        ptile[:], xT[:], AF.Copy, bias=0.0, scale=0.01
                    )
                else:
                    for h in range(halves):
                        emit_plane(j, h, ptile)
                if ablate != "mm":
                    for c in range(nchunk):
                        for m in range(nbt):
                            lhsT = ptile[:, c * nb + m * P : c * nb + (m + 1) * P]
                            for n in range(nnt):
                                nc.tensor.matmul(
                                    psum_o[(m, n)][:],
                                    lhsT,
                                    wts[c][:, n * NT : (n + 1) * NT],
                                    start=(j == 0 and c == 0),
                                    stop=(j == NJ - 1 and c == nchunk - 1),
                                )

            if ablate == "mm":
                return
            for m in range(nbt):
                for n in range(nnt):
                    ob = outp.tile([P, NT], f32, tag="ob", name=f"ob{m}_{n}")
                    nc.vector.tensor_copy(ob[:], psum_o[(m, n)][:])
                    nc.sync.dma_start(
                        out[m * P : (m + 1) * P, n * NT : (n + 1) * NT], ob[:]
                    )

        if repeat is None:
            body()
        else:
            with tc.For_i(0, repeat, 1):
                body()

    if compile:
        nc.compile()
    return nc


_PROG = None


def _get_prog():
    global _PROG
    if _PROG is None:
        _PROG = build_program()
    return _PROG


def kernel(x, spline_kernel):
    from concourse.bass_utils import run_bass_kernel_spmd

    nc = _get_prog()
    x = np.ascontiguousarray(np.asarray(x, dtype=np.float32))
    w = np.ascontiguousarray(np.asarray(spline_kernel, dtype=np.float32))
    nb = BATCH // N_CORES
    in_maps = [{"x": x[c * nb : (c + 1) * nb], "w": w} for c in range(N_CORES)]
    res = run_bass_kernel_spmd(nc, in_maps, core_ids=list(range(N_CORES)))
    return np.concatenate([res.results[c]["out"] for c in range(N_CORES)], axis=0)
